# revision 1
# baseline (speedup 1.0000x reference)
"""Trainium2 Bass kernel for MinimalLBS (B=32, T=128, N=2048, J=52, Jb=21, L=16).

Data-parallel over B across 8 NeuronCores (4 samples per core).

Device math per sample (per 128-vertex chunk, t free):
  MAIN (exact, bf16 on PE): fold v_template+homogeneous into the stationary:
     M[n,(i,t)] = sum_{(j,k)} wvh[(j,k),n] * arm[(j,k),(i,t)]
     where wvh[(j,k),n] = wt[k,n]*vth[n,j]  (host-prepped, K=212),
     arm[(j,k),(i,t)] = A[t,k,i,j] (+ translation row at (3,J)).
     This covers Ts @ [v_template,1] + trans exactly - no per-vertex matvec.
  CORRECTION (fp8 DoubleRow): dv[n,j,t] (pose+shape offsets, K=206 dual-tile)
     and ts8[n,(j,i,t)] (K=52 dual-tile) on PE; pm8 = ts8*dv on DVE/Pool
     (fp8 out, x32 scale in D8); j-reduction via fp8 identity matmuls
     (ident/32) accumulating straight into the M PSUM bank.
  One ACT evac per chunk into a per-sample out buffer; one output DMA/sample.
"""

import sys

sys.path.insert(0, "/opt/trn_rl_repo")

import math

import ml_dtypes
import numpy as np

import concourse.bacc as bacc
import concourse.bass as bass
import concourse.mybir as mybir
import concourse.tile as tile
from concourse import bass_utils, masks

F32 = mybir.dt.float32
BF16 = mybir.dt.bfloat16
FP8 = mybir.dt.float8e4
NPBF16 = ml_dtypes.bfloat16
NPF8 = ml_dtypes.float8_e4m3

B, T, N, JB, J, L = 32, 128, 2048, 21, 52, 16
NCORES = 8
NB = B // NCORES          # samples per core
PF = JB * 9               # 189 pose-feature dims
Z = PF + L                # 205 combined correction coeffs
KA = (J + 1) * 4          # 212 (j,k) rows for the main matmul
NCH = N // 128            # n-chunks per sample
DS = 32.0                 # fp8 scale baked into D8, undone by ident/DS

_CACHED = {}


def _build_nc():
    nc = bacc.Bacc("TRN2", target_bir_lowering=False, debug=False)

    pose_d = nc.dram_tensor("pose", [T, NB, JB, 3], F32, kind="ExternalInput")
    betasb_d = nc.dram_tensor("betasb", [T, NB, L], BF16, kind="ExternalInput")
    wvha_d = nc.dram_tensor("wvha", [128, NB, N], BF16, kind="ExternalInput")
    wvhb_d = nc.dram_tensor("wvhb", [KA - 128, NB, N], BF16, kind="ExternalInput")
    arma_d = nc.dram_tensor("arma", [128, NB, 3 * T], BF16, kind="ExternalInput")
    armb_d = nc.dram_tensor("armb", [KA - 128, NB, 3 * T], BF16,
                            kind="ExternalInput")
    wt8_d = nc.dram_tensor("wt8", [26, 2, NB, N], FP8, kind="ExternalInput")
    ar8_d = nc.dram_tensor("ar8", [26, 2, NB, 3, 3, T], FP8, kind="ExternalInput")
    d8_d = nc.dram_tensor("d8", [NB, 128, 2, 3, N], FP8, kind="ExternalInput")
    out_d = nc.dram_tensor("out", [NB, NCH, 128, 3 * T], BF16,
                           kind="ExternalOutput")

    with tile.TileContext(nc) as tc:
        with (
            tc.tile_pool(name="const", bufs=1) as p_const,
            tc.tile_pool(name="rod", bufs=1) as p_rod,
            tc.tile_pool(name="glob", bufs=1) as p_glob,
            tc.tile_pool(name="samp", bufs=2) as p_samp,
            tc.tile_pool(name="work", bufs=2) as p_work,
            tc.tile_pool(name="psm", bufs=2, space="PSUM") as ps_m,
            tc.tile_pool(name="psts", bufs=2, space="PSUM") as ps_ts,
        ):

            # ---- input DMAs: pose first (unblocks Rodrigues), then the
            # correction-path tensors (gate the first chunks), then main-path
            # per-sample slices (prefetched one sample ahead in the loop).
            po = p_rod.tile([T, NB, JB, 3], F32)
            nc.sync.dma_start(po[:], pose_d[:])
            zcf = p_rod.tile([T, NB, 256], BF16)
            nc.sync.dma_start(zcf[:, :, PF:Z], betasb_d[:])
            nc.vector.memset(zcf[:, :, Z:256], 0.0)
            d8_t = [p_glob.tile([128, 2, 3, N], FP8, tag=f"d8_{i}",
                                name=f"d8_{i}") for i in range(NB)]
            nc.sync.dma_start(d8_t[0][:], d8_d[0])
            wt8 = p_glob.tile([26, 2, NB, N], FP8)
            nc.sync.dma_start(wt8[:], wt8_d[:])
            ar8 = p_glob.tile([26, 2, NB, 3, 3, T], FP8)
            nc.sync.dma_start(ar8[:], ar8_d[:])
            wvha_t = [p_glob.tile([128, N], BF16, tag=f"wvha_{i}",
                                  name=f"wvha_{i}") for i in range(NB)]
            wvhb_t = [p_glob.tile([KA - 128, N], BF16, tag=f"wvhb_{i}",
                                  name=f"wvhb_{i}") for i in range(NB)]
            arma_t = [p_glob.tile([128, 3 * T], BF16, tag=f"arma_{i}",
                                  name=f"arma_{i}") for i in range(NB)]
            armb_t = [p_glob.tile([KA - 128, 3 * T], BF16, tag=f"armb_{i}",
                                  name=f"armb_{i}") for i in range(NB)]

            def main_dmas(nb):
                nc.sync.dma_start(wvha_t[nb][:], wvha_d[:, nb])
                nc.sync.dma_start(arma_t[nb][:], arma_d[:, nb])
                nc.sync.dma_start(wvhb_t[nb][:], wvhb_d[:, nb])
                nc.sync.dma_start(armb_t[nb][:], armb_d[:, nb])
            # ---- Rodrigues via polynomials (no sqrt/sin/transcendental):
            # with x = |aa|^2, g1 = sin(sqrt(x))/sqrt(x), g2 = (1-cos(sqrt(x)))/x
            # are entire functions; (R-I) entries are polynomial in aa given
            # g1, g2: diag_i = g2*(aa_i^2 - x), offdiag = g2*aa_i*aa_j +- g1*aa_k.
            # Emitted per-sample: sample 0 on DVE (short critical chain into
            # the first transposes), samples 1..3 on the otherwise-idle Pool
            # (hidden under steady-state compute).
            identb = p_const.tile([128, 128], BF16)
            masks.make_identity(nc, identb[:])
            ident8 = p_const.tile([128, 128], FP8)
            nc.scalar.activation(ident8[:], identb[:],
                                 mybir.ActivationFunctionType.Identity,
                                 scale=1.0 / DS)
            ident8dr = p_const.tile([128, 2, 128], FP8)
            nc.scalar.copy(ident8dr[:, 0, :], ident8[:])
            nc.scalar.copy(ident8dr[:, 1, :], ident8[:])
            pf = zcf[:, :, 0:PF].rearrange("t nb (j e) -> t nb j e", j=JB)
            M_, A_ = mybir.AluOpType.mult, mybir.AluOpType.add
            S_ = mybir.AluOpType.subtract
            with tc.high_priority():
                sq = p_rod.tile([T, NB, JB, 3], F32)
                nc.vector.tensor_tensor(sq[:], po[:], po[:], M_)
                a2 = p_rod.tile([T, NB, JB], F32)
                nc.vector.tensor_tensor(a2[:], sq[:, :, :, 0],
                                        sq[:, :, :, 1], A_)
                x = p_rod.tile([T, NB, JB], F32)
                nc.vector.tensor_tensor(x[:], a2[:], sq[:, :, :, 2], A_)
                xx = p_rod.tile([T, NB, JB], F32)
                nc.vector.tensor_tensor(xx[:], x[:], x[:], M_)
                t1 = p_rod.tile([T, NB, JB], F32)
                nc.vector.tensor_scalar(t1[:], x[:], -1.0 / 6, 1.0, M_, A_)
                h1 = p_rod.tile([T, NB, JB], F32)
                nc.vector.tensor_scalar(h1[:], x[:], -1.0 / 5040, 1.0 / 120,
                                        M_, A_)
                h1b = p_rod.tile([T, NB, JB], F32)
                nc.vector.tensor_tensor(h1b[:], h1[:], xx[:], M_)
                g1 = p_rod.tile([T, NB, JB], F32)
                nc.vector.tensor_tensor(g1[:], h1b[:], t1[:], A_)
                t3 = p_rod.tile([T, NB, JB], F32)
                nc.vector.tensor_scalar(t3[:], x[:], -1.0 / 24, 0.5, M_, A_)
                h2 = p_rod.tile([T, NB, JB], F32)
                nc.vector.tensor_scalar(h2[:], x[:], -1.0 / 40320, 1.0 / 720,
                                        M_, A_)
                h2b = p_rod.tile([T, NB, JB], F32)
                nc.vector.tensor_tensor(h2b[:], h2[:], xx[:], M_)
                g2 = p_rod.tile([T, NB, JB], F32)
                nc.vector.tensor_tensor(g2[:], h2b[:], t3[:], A_)

                prods = {}
                for (a, b2), nm in [((0, 1), "xy"), ((0, 2), "xz"),
                                    ((1, 2), "yz")]:
                    t_ = p_rod.tile([T, NB, JB], F32, tag=f"pr_{nm}",
                                    name=f"pr_{nm}")
                    nc.vector.tensor_tensor(t_[:], po[:, :, :, a],
                                            po[:, :, :, b2], M_)
                    prods[nm] = t_
                qs = {}
                for i, nm in [(0, "qx"), (1, "qy"), (2, "qz")]:
                    t_ = p_rod.tile([T, NB, JB], F32, tag=f"q_{nm}",
                                    name=f"q_{nm}")
                    nc.vector.tensor_tensor(t_[:], g1[:], po[:, :, :, i], M_)
                    qs[nm] = t_
                os_ = {}
                for nm in ["xy", "xz", "yz"]:
                    t_ = p_rod.tile([T, NB, JB], F32, tag=f"o_{nm}",
                                    name=f"o_{nm}")
                    nc.vector.tensor_tensor(t_[:], g2[:], prods[nm][:], M_)
                    os_[nm] = t_
                for di, c in [(0, 0), (4, 1), (8, 2)]:
                    d_ = p_rod.tile([T, NB, JB], F32, tag=f"d_{di}",
                                    name=f"d_{di}")
                    nc.vector.tensor_tensor(d_[:], sq[:, :, :, c], x[:], S_)
                    nc.vector.tensor_tensor(pf[:, :, :, di], g2[:], d_[:], M_)
                for e, o_nm, q_nm, op in [
                    (1, "xy", "qz", S_), (3, "xy", "qz", A_),
                    (2, "xz", "qy", A_), (6, "xz", "qy", S_),
                    (5, "yz", "qx", S_), (7, "yz", "qx", A_),
                ]:
                    nc.vector.tensor_tensor(pf[:, :, :, e], os_[o_nm][:],
                                            qs[q_nm][:], op)

            zc8_t = [None] * NB
            zcT_t = [None] * NB
            outacc_t = [None] * NB

            def setup_transposes(nb):
                # zcT [128, 2, T]: k-tile0 = zc rows 0:128, k-tile1 = rows
                # 128:206 + zero pad (cols 205:256 of zcf are zero).
                zc_nb = zcf[:, nb]
                zcT = p_samp.tile([128, 2, T], BF16, tag="zcT", name="zcT")
                nc.sync.dma_start_transpose(zcT[:, 0, :], zc_nb[:, 0:128])
                nc.sync.dma_start_transpose(zcT[:, 1, :], zc_nb[:, 128:256])
                zcT_t[nb] = zcT

            def setup_convert(nb):
                zc8 = p_samp.tile([128, 2, T], FP8, tag="zc8", name="zc8")
                nc.scalar.copy(zc8[:], zcT_t[nb][:])
                zc8_t[nb] = zc8
                outacc_t[nb] = p_samp.tile([128, NCH, 3 * T], BF16,
                                           tag="outacc", name="outacc")

            NTOT = NB * NCH
            OUTB = 4  # chunks per output DMA burst
            pend = []  # staged chunks: pm8 -> (main+reduce) -> evac
            for gi in range(NTOT + 3):
                if gi < NTOT:
                    nb, nch = divmod(gi, NCH)
                    if gi == 0:
                        setup_transposes(0)
                        setup_convert(0)
                        main_dmas(0)
                    if nch == 2 and nb + 1 < NB:
                        nc.sync.dma_start(d8_t[nb + 1][:], d8_d[nb + 1])
                        setup_transposes(nb + 1)
                    if nch == 10 and nb + 1 < NB:
                        main_dmas(nb + 1)
                        setup_convert(nb + 1)
                    n0 = nch * 128
                    nsl = slice(n0, n0 + 128)

                    # cb [128, 3(j), 512] f32: cols 0:384 = ts8 slab (i,t),
                    # cols 384:512 = dv (the otherwise-wasted bank pad).
                    cb = ps_ts.tile([128, 3, 512], F32, tag="ts8")
                    with tc.high_priority():
                        for j in range(3):
                            nc.tensor.matmul(
                                cb[:, j, 384:512], d8_t[nb][:, :, j, nsl],
                                zc8_t[nb][:],
                                start=True, stop=True,
                                perf_mode=mybir.MatmulPerfMode.DoubleRow,
                            )
                        for j in range(3):
                            nc.tensor.matmul(
                                cb[:, j, 0:384], wt8[:, :, nb, nsl],
                                ar8[:, :, nb, j].rearrange(
                                    "k u i t -> k u (i t)"),
                                start=True, stop=True,
                                perf_mode=mybir.MatmulPerfMode.DoubleRow,
                            )
                    # dv evac PSUM->SBUF bf16 (both-PSUM tt illegal).
                    # high_priority: ACT must always prefer this over output
                    # evacs, since pm (the DVE bottleneck) waits on it.
                    dvs = p_work.tile([128, 3, T], BF16, tag="dvs")
                    with tc.high_priority():
                        nc.scalar.copy(dvs[:], cb[:, :, 384:512])

                if pend and pend[-1][4] == 0:
                    # main + j-reduce of chunk gi-1 (PE: after chunk gi's cb
                    # matmuls so cb(gi) is ready long before pm(gi) needs it)
                    _, ppm8, pnb, pnch, _st = pend[-1]
                    pM = ps_m.tile([128, 3 * T], F32, tag="M")
                    pnsl = slice(pnch * 128, pnch * 128 + 128)
                    nc.tensor.matmul(pM[:], wvha_t[pnb][:, pnsl],
                                     arma_t[pnb][:], start=True, stop=False)
                    nc.tensor.matmul(pM[:], wvhb_t[pnb][:, pnsl],
                                     armb_t[pnb][:], start=False, stop=False)
                    nc.tensor.matmul(
                        pM[:], ident8dr[:],
                        ppm8[:, 0:2].rearrange("n j i t -> n j (i t)"),
                        start=False, stop=False,
                        perf_mode=mybir.MatmulPerfMode.DoubleRow,
                        skip_group_check=True,
                    )
                    nc.tensor.matmul(
                        pM[:], ident8[:],
                        ppm8[:, 2].rearrange("n i t -> n (i t)"),
                        start=False, stop=True, skip_group_check=True,
                    )
                    pend[-1] = [gi - 1, pM, pnb, pnch, 1]

                if pend and pend[0][4] == 1 and (gi - pend[0][0] >= 2
                                                 or gi >= NTOT):
                    # evac of chunk gi-2 (ACT: two behind, so the evac's
                    # reduce-wait never delays the next dvs in ACT's queue)
                    _, pM2, pnb2, pnch2, _st = pend.pop(0)
                    nc.scalar.copy(outacc_t[pnb2][:, pnch2, :], pM2[:])
                    ob = 2 if pnb2 == NB - 1 and pnch2 >= NCH - 4 else OUTB
                    if pnch2 % ob == ob - 1:
                        c0 = pnch2 - (ob - 1)
                        nc.sync.dma_start(
                            out_d[pnb2, c0:pnch2 + 1].rearrange(
                                "c p f -> p c f"),
                            outacc_t[pnb2][:, c0:pnch2 + 1],
                        )

                if gi < NTOT:
                    # pm8 [128, 3(j), 3(i), T] fp8 = ts8 * dvs (DVE)
                    pm8 = p_work.tile([128, 3, 3, T], FP8, tag="pm8")
                    nc.vector.tensor_tensor(
                        pm8[:],
                        cb[:, :, 0:384].rearrange(
                            "n j (i t) -> n j i t", i=3),
                        dvs[:].unsqueeze(2).broadcast_to((128, 3, 3, T)),
                        mybir.AluOpType.mult,
                    )
                    pend.append([gi, pm8, nb, nch, 0])

    nc.compile()
    return nc


def _prep_core(c, pose_body, trans, betas, A, v_template, shapedirs, posedirs,
               lbs_weights):
    bs = slice(NB * c, NB * (c + 1))
    pose = np.ascontiguousarray(
        pose_body[bs].transpose(1, 0, 2).reshape(T, NB, JB, 3)
    ).astype(np.float32)

    betasb = np.ascontiguousarray(
        betas[bs].transpose(1, 0, 2)
    ).astype(NPBF16)                                           # [T, NB, L]

    wt = np.concatenate(
        [lbs_weights[bs].transpose(0, 2, 1),
         np.ones((NB, 1, N), np.float32)], axis=1)             # [NB, 53, N]
    vth = np.concatenate(
        [v_template[bs], np.ones((NB, N, 1), np.float32)], axis=2)  # [NB,N,4]
    wvh = (vth.transpose(0, 2, 1)[:, :, None, :] * wt[:, None, :, :]
           ).reshape(NB, KA, N)                                # [NB,(j,k),N]
    wvha = np.ascontiguousarray(wvh[:, 0:128].transpose(1, 0, 2)).astype(NPBF16)
    wvhb = np.ascontiguousarray(wvh[:, 128:KA].transpose(1, 0, 2)).astype(NPBF16)

    arm = np.zeros((NB, 4, J + 1, 3, T), np.float32)
    arm[:, :, :J] = A[bs, :, :, 0:3, :].transpose(0, 4, 2, 3, 1)  # [nb,j,k,i,t]
    arm[:, 3, J] = trans[bs].transpose(0, 2, 1)                   # [nb,i,t]
    arm = arm.reshape(NB, KA, 3 * T)
    arma = np.ascontiguousarray(arm[:, 0:128].transpose(1, 0, 2)).astype(NPBF16)
    armb = np.ascontiguousarray(arm[:, 128:KA].transpose(1, 0, 2)).astype(NPBF16)

    wt8 = np.empty((26, 2, NB, N), np.float32)
    wt8[:, 0] = wt[:, 0:26].transpose(1, 0, 2)
    wt8[:, 1] = wt[:, 26:52].transpose(1, 0, 2)
    wt8 = wt8.astype(NPF8)

    ar8f = A[bs, :, :, 0:3, 0:3].transpose(0, 2, 4, 3, 1)      # [nb,k,j,i,t]
    ar8 = np.empty((26, 2, NB, 3, 3, T), np.float32)
    ar8[:, 0] = ar8f[:, 0:26].transpose(1, 0, 2, 3, 4)
    ar8[:, 1] = ar8f[:, 26:52].transpose(1, 0, 2, 3, 4)
    ar8 = ar8.astype(NPF8)

    D = np.concatenate([
        posedirs[bs].reshape(NB, PF, N, 3),
        shapedirs[bs].transpose(0, 3, 1, 2),                   # [NB, L, N, 3]
    ], axis=1)                                                 # [NB, 205, N, 3]
    Dt = D.transpose(0, 1, 3, 2) * DS                          # [NB, 205, 3, N]
    d8 = np.zeros((NB, 128, 2, 3, N), np.float32)
    d8[:, :, 0] = Dt[:, 0:128]
    d8[:, 0:77, 1] = Dt[:, 128:205]
    d8 = d8.astype(NPF8)

    return {
        "pose": pose, "betasb": betasb, "wvha": wvha, "wvhb": wvhb,
        "arma": arma, "armb": armb, "wt8": wt8, "ar8": ar8, "d8": d8,
    }


def kernel(pose_body, trans, betas, A, v_template, shapedirs, posedirs,
           lbs_weights):
    if "nc" not in _CACHED:
        _CACHED["nc"] = _build_nc()
    nc = _CACHED["nc"]

    args = (pose_body, trans, betas, A, v_template, shapedirs, posedirs,
            lbs_weights)
    args = tuple(np.asarray(a, dtype=np.float32) for a in args)
    in_maps = [_prep_core(c, *args) for c in range(NCORES)]

    res = bass_utils.run_bass_kernel_spmd(nc, in_maps,
                                          core_ids=list(range(NCORES)))

    # out [NB, NCH, 128, 3*T] per core -> (B, T, N, 3)
    full = np.stack(
        [res.results[c]["out"].astype(np.float32) for c in range(NCORES)]
    )
    full = full.reshape(B, NCH, 128, 3, T).transpose(0, 4, 1, 2, 3)
    return np.ascontiguousarray(full.reshape(B, T, N, 3).astype(np.float32))



# revision 3
# speedup vs baseline: 1.0123x; 1.0123x over previous
"""Trainium2 Bass kernel for MinimalLBS (B=32, T=128, N=2048, J=52, Jb=21, L=16).

Data-parallel over B across 8 NeuronCores (4 samples per core).

Device math per sample (per 128-vertex chunk, t free):
  MAIN (exact, bf16 on PE): fold v_template+homogeneous into the stationary:
     M[n,(i,t)] = sum_{(j,k)} wvh[(j,k),n] * arm[(j,k),(i,t)]
     where wvh[(j,k),n] = wt[k,n]*vth[n,j]  (host-prepped, K=212),
     arm[(j,k),(i,t)] = A[t,k,i,j] (+ translation row at (3,J)).
     This covers Ts @ [v_template,1] + trans exactly - no per-vertex matvec.
  CORRECTION (fp8 DoubleRow): dv[n,j,t] (pose+shape offsets, K=206 dual-tile
     over 103 partitions, zero-pad row handled in zc8) and ts8[n,(j,i,t)]
     (K=52 dual-tile) on PE; pm8 = ts8*dv on DVE (fp8 out, x32 scale in D8);
     j-reduction via fp8 identity matmuls (ident/32) accumulating straight
     into the M PSUM bank.
  One ACT evac per chunk into a per-sample out buffer; one output DMA/sample.

Rodrigues pose features + betas are folded on the HOST into zc8 (fp8
[103, 2, NB, T]) - no on-device Rodrigues, transposes, or converts.  PE is
pre-warmed with dummy matmuls so real matmuls run at the full 2.4GHz pstate.
"""

import sys

sys.path.insert(0, "/opt/trn_rl_repo")

import ml_dtypes
import numpy as np

import concourse.bacc as bacc
import concourse.mybir as mybir
import concourse.tile as tile
from concourse import bass_utils, masks

F32 = mybir.dt.float32
BF16 = mybir.dt.bfloat16
FP8 = mybir.dt.float8e4
NPBF16 = ml_dtypes.bfloat16
NPF8 = ml_dtypes.float8_e4m3

B, T, N, JB, J, L = 32, 128, 2048, 21, 52, 16
NCORES = 8
NB = B // NCORES          # samples per core
PF = JB * 9               # 189 pose-feature dims
Z = PF + L                # 205 combined correction coeffs
ZP = 103                  # dual-pair partitions for the z contraction (206>=205)
KA = (J + 1) * 4          # 212 (j,k) rows for the main matmul
NCH = N // 128            # n-chunks per sample
DS = 32.0                 # fp8 scale baked into D8, undone by ident/DS

_CACHED = {}


def _build_nc():
    nc = bacc.Bacc("TRN2", target_bir_lowering=False, debug=False)

    zc8_d = nc.dram_tensor("zc8", [ZP, 2, NB, T], FP8, kind="ExternalInput")
    wvha_d = nc.dram_tensor("wvha", [128, NB, N], BF16, kind="ExternalInput")
    wvhb_d = nc.dram_tensor("wvhb", [KA - 128, NB, N], BF16, kind="ExternalInput")
    arma_d = nc.dram_tensor("arma", [128, NB, 3 * T], BF16, kind="ExternalInput")
    armb_d = nc.dram_tensor("armb", [KA - 128, NB, 3 * T], BF16,
                            kind="ExternalInput")
    wt8_d = nc.dram_tensor("wt8", [26, 2, NB, N], FP8, kind="ExternalInput")
    ar8_d = nc.dram_tensor("ar8", [26, 2, NB, 3, 3, T], FP8, kind="ExternalInput")
    d8_d = nc.dram_tensor("d8", [NB, ZP, 2, 3, N], FP8, kind="ExternalInput")
    out_d = nc.dram_tensor("out", [NB, NCH, 128, 3 * T], BF16,
                           kind="ExternalOutput")

    with tile.TileContext(nc) as tc:
        with (
            tc.tile_pool(name="const", bufs=1) as p_const,
            tc.tile_pool(name="glob", bufs=1) as p_glob,
            tc.tile_pool(name="samp", bufs=2) as p_samp,
            tc.tile_pool(name="work", bufs=3) as p_work,
            tc.tile_pool(name="psm", bufs=2, space="PSUM") as ps_m,
            tc.tile_pool(name="psts", bufs=2, space="PSUM") as ps_ts,
        ):

            # ---- input DMAs: correction-path tensors first (gate the first
            # chunks), then main-path per-sample slices (prefetched one sample
            # ahead in the loop).
            zc8 = p_glob.tile([ZP, 2, NB, T], FP8)
            nc.sync.dma_start(zc8[:], zc8_d[:])
            d8_t = [p_glob.tile([ZP, 2, 3, N], FP8, tag=f"d8_{i}",
                                name=f"d8_{i}") for i in range(NB)]
            nc.sync.dma_start(d8_t[0][:], d8_d[0])
            wt8 = p_glob.tile([26, 2, NB, N], FP8)
            nc.sync.dma_start(wt8[:], wt8_d[:])
            ar8 = p_glob.tile([26, 2, NB, 3, 3, T], FP8)
            nc.sync.dma_start(ar8[:], ar8_d[:])
            wvha_t = [p_glob.tile([128, N], BF16, tag=f"wvha_{i}",
                                  name=f"wvha_{i}") for i in range(NB)]
            wvhb_t = [p_glob.tile([KA - 128, N], BF16, tag=f"wvhb_{i}",
                                  name=f"wvhb_{i}") for i in range(NB)]
            arma_t = [p_glob.tile([128, 3 * T], BF16, tag=f"arma_{i}",
                                  name=f"arma_{i}") for i in range(NB)]
            armb_t = [p_glob.tile([KA - 128, 3 * T], BF16, tag=f"armb_{i}",
                                  name=f"armb_{i}") for i in range(NB)]

            def main_dmas(nb):
                nc.sync.dma_start(wvha_t[nb][:], wvha_d[:, nb])
                nc.sync.dma_start(arma_t[nb][:], arma_d[:, nb])
                nc.sync.dma_start(wvhb_t[nb][:], wvhb_d[:, nb])
                nc.sync.dma_start(armb_t[nb][:], armb_d[:, nb])

            identb = p_const.tile([128, 128], BF16)
            masks.make_identity(nc, identb[:])
            ident8 = p_const.tile([128, 128], FP8)
            nc.scalar.activation(ident8[:], identb[:],
                                 mybir.ActivationFunctionType.Identity,
                                 scale=1.0 / DS)
            ident8dr = p_const.tile([128, 2, 128], FP8)
            nc.scalar.copy(ident8dr[:, 0, :], ident8[:])
            nc.scalar.copy(ident8dr[:, 1, :], ident8[:])

            # ---- PE pre-warm: ~60 dummy DR matmuls (junk into the first pM
            # pool buffer, overwritten by chunk 0's start=True mains) so the
            # 3us pstate ramp completes while the input DMAs stream in.
            pm_warm = ps_m.tile([128, 3 * T], F32, tag="M")
            for _ in range(60):
                nc.tensor.matmul(pm_warm[:, 0:128], ident8dr[:], ident8dr[:],
                                 start=True, stop=True,
                                 perf_mode=mybir.MatmulPerfMode.DoubleRow)

            outacc_t = [None] * NB

            NTOT = NB * NCH
            OUTB = 4  # chunks per output DMA burst
            pend = []  # staged chunks: pm8 -> (main+reduce) -> evac
            for gi in range(NTOT + 3):
                if gi < NTOT:
                    nb, nch = divmod(gi, NCH)
                    if gi == 0:
                        main_dmas(0)
                        outacc_t[0] = p_samp.tile([128, NCH, 3 * T], BF16,
                                                  tag="outacc", name="outacc")
                    if nch == 2 and nb + 1 < NB:
                        nc.sync.dma_start(d8_t[nb + 1][:], d8_d[nb + 1])
                    if nch == 10 and nb + 1 < NB:
                        main_dmas(nb + 1)
                        outacc_t[nb + 1] = p_samp.tile(
                            [128, NCH, 3 * T], BF16, tag="outacc",
                            name="outacc")
                    n0 = nch * 128
                    nsl = slice(n0, n0 + 128)

                    # cb [128, 3(j), 512] f32: cols 0:384 = ts8 slab (i,t),
                    # cols 384:512 = dv (the otherwise-wasted bank pad).
                    cb = ps_ts.tile([128, 3, 512], F32, tag="ts8")
                    with tc.high_priority():
                        for j in range(3):
                            nc.tensor.matmul(
                                cb[:, j, 384:512], d8_t[nb][:, :, j, nsl],
                                zc8[:, :, nb, :],
                                start=True, stop=True,
                                perf_mode=mybir.MatmulPerfMode.DoubleRow,
                            )
                    # dv evac PSUM->SBUF bf16 (both-PSUM tt illegal), emitted
                    # before the ts8 matmuls so its semaphore lands well ahead
                    # of pm8's issue.  high_priority: ACT must always prefer
                    # this over output evacs, since pm (the DVE bottleneck)
                    # waits on it.
                    dvs = p_work.tile([128, 3, T], BF16, tag="dvs")
                    with tc.high_priority():
                        nc.scalar.copy(dvs[:], cb[:, :, 384:512])
                        for j in range(3):
                            nc.tensor.matmul(
                                cb[:, j, 0:384], wt8[:, :, nb, nsl],
                                ar8[:, :, nb, j].rearrange(
                                    "k u i t -> k u (i t)"),
                                start=True, stop=True,
                                perf_mode=mybir.MatmulPerfMode.DoubleRow,
                            )

                if pend and pend[-1][4] == 0:
                    # main + j-reduce of chunk gi-1 (PE: after chunk gi's cb
                    # matmuls so cb(gi) is ready long before pm(gi) needs it)
                    _, ppm8, pnb, pnch, _st = pend[-1]
                    pM = ps_m.tile([128, 3 * T], F32, tag="M")
                    pnsl = slice(pnch * 128, pnch * 128 + 128)
                    nc.tensor.matmul(pM[:], wvha_t[pnb][:, pnsl],
                                     arma_t[pnb][:], start=True, stop=False)
                    nc.tensor.matmul(pM[:], wvhb_t[pnb][:, pnsl],
                                     armb_t[pnb][:], start=False, stop=False)
                    nc.tensor.matmul(
                        pM[:], ident8dr[:],
                        ppm8[:, 0:2].rearrange("n j i t -> n j (i t)"),
                        start=False, stop=False,
                        perf_mode=mybir.MatmulPerfMode.DoubleRow,
                        skip_group_check=True,
                    )
                    nc.tensor.matmul(
                        pM[:], ident8[:],
                        ppm8[:, 2].rearrange("n i t -> n (i t)"),
                        start=False, stop=True, skip_group_check=True,
                    )
                    pend[-1] = [gi - 1, pM, pnb, pnch, 1]

                if pend and pend[0][4] == 1 and (gi - pend[0][0] >= 2
                                                 or gi >= NTOT):
                    # evac of chunk gi-2 (ACT: two behind, so the evac's
                    # reduce-wait never delays the next dvs in ACT's queue)
                    _, pM2, pnb2, pnch2, _st = pend.pop(0)
                    nc.scalar.copy(outacc_t[pnb2][:, pnch2, :], pM2[:])
                    ob = 2 if pnb2 == NB - 1 and pnch2 >= NCH - 4 else OUTB
                    if pnch2 % ob == ob - 1:
                        c0 = pnch2 - (ob - 1)
                        nc.sync.dma_start(
                            out_d[pnb2, c0:pnch2 + 1].rearrange(
                                "c p f -> p c f"),
                            outacc_t[pnb2][:, c0:pnch2 + 1],
                        )

                if gi < NTOT:
                    # pm8 [128, 3(j), 3(i), T] fp8 = ts8 * dvs (DVE)
                    pm8 = p_work.tile([128, 3, 3, T], FP8, tag="pm8")
                    nc.vector.tensor_tensor(
                        pm8[:],
                        cb[:, :, 0:384].rearrange(
                            "n j (i t) -> n j i t", i=3),
                        dvs[:].unsqueeze(2).broadcast_to((128, 3, 3, T)),
                        mybir.AluOpType.mult,
                    )
                    pend.append([gi, pm8, nb, nch, 0])

    nc.compile()
    return nc


def _rodrigues_feat(pose):
    # pose: [NB, T, JB, 3] float32 -> (R - I) flattened [NB, T, PF]
    aa = pose.astype(np.float32)
    angle = np.sqrt((aa * aa).sum(-1, keepdims=True))            # [NB,T,JB,1]
    axis = aa / np.maximum(angle, 1e-8)
    x, y, z = axis[..., 0], axis[..., 1], axis[..., 2]
    s = np.sin(angle[..., 0])[..., None, None]
    c = np.cos(angle[..., 0])[..., None, None]
    zero = np.zeros_like(x)
    K = np.stack([
        np.stack([zero, -z, y], axis=-1),
        np.stack([z, zero, -x], axis=-1),
        np.stack([-y, x, zero], axis=-1),
    ], axis=-2)
    outer = axis[..., :, None] * axis[..., None, :]
    I = np.eye(3, dtype=np.float32)
    R = c * I + s * K + (1.0 - c) * outer
    return (R - I).reshape(aa.shape[0], aa.shape[1], PF)


def _prep_core(c, pose_body, trans, betas, A, v_template, shapedirs, posedirs,
               lbs_weights):
    bs = slice(NB * c, NB * (c + 1))

    # zc = [pose_feature | betas] per (sample, t), packed into fp8 dual-pairs
    # (z, z+103); the phantom row z=205 is the zero at (102, u=1).
    pf = _rodrigues_feat(pose_body[bs].reshape(NB, T, JB, 3))    # [NB,T,PF]
    zc = np.concatenate([pf, betas[bs]], axis=2)                 # [NB,T,Z]
    zcT = np.ascontiguousarray(zc.transpose(2, 0, 1))            # [Z,NB,T]
    zc8 = np.zeros((ZP, 2, NB, T), np.float32)
    zc8[:, 0] = zcT[0:ZP]
    zc8[0:Z - ZP, 1] = zcT[ZP:Z]
    zc8 = zc8.astype(NPF8)

    wt = np.concatenate(
        [lbs_weights[bs].transpose(0, 2, 1),
         np.ones((NB, 1, N), np.float32)], axis=1)             # [NB, 53, N]
    vth = np.concatenate(
        [v_template[bs], np.ones((NB, N, 1), np.float32)], axis=2)  # [NB,N,4]
    wvh = (vth.transpose(0, 2, 1)[:, :, None, :] * wt[:, None, :, :]
           ).reshape(NB, KA, N)                                # [NB,(j,k),N]
    wvha = np.ascontiguousarray(wvh[:, 0:128].transpose(1, 0, 2)).astype(NPBF16)
    wvhb = np.ascontiguousarray(wvh[:, 128:KA].transpose(1, 0, 2)).astype(NPBF16)

    arm = np.zeros((NB, 4, J + 1, 3, T), np.float32)
    arm[:, :, :J] = A[bs, :, :, 0:3, :].transpose(0, 4, 2, 3, 1)  # [nb,j,k,i,t]
    arm[:, 3, J] = trans[bs].transpose(0, 2, 1)                   # [nb,i,t]
    arm = arm.reshape(NB, KA, 3 * T)
    arma = np.ascontiguousarray(arm[:, 0:128].transpose(1, 0, 2)).astype(NPBF16)
    armb = np.ascontiguousarray(arm[:, 128:KA].transpose(1, 0, 2)).astype(NPBF16)

    wt8 = np.empty((26, 2, NB, N), np.float32)
    wt8[:, 0] = wt[:, 0:26].transpose(1, 0, 2)
    wt8[:, 1] = wt[:, 26:52].transpose(1, 0, 2)
    wt8 = wt8.astype(NPF8)

    ar8f = A[bs, :, :, 0:3, 0:3].transpose(0, 2, 4, 3, 1)      # [nb,k,j,i,t]
    ar8 = np.empty((26, 2, NB, 3, 3, T), np.float32)
    ar8[:, 0] = ar8f[:, 0:26].transpose(1, 0, 2, 3, 4)
    ar8[:, 1] = ar8f[:, 26:52].transpose(1, 0, 2, 3, 4)
    ar8 = ar8.astype(NPF8)

    D = np.concatenate([
        posedirs[bs].reshape(NB, PF, N, 3),
        shapedirs[bs].transpose(0, 3, 1, 2),                   # [NB, L, N, 3]
    ], axis=1)                                                 # [NB, Z, N, 3]
    Dt = D.transpose(0, 1, 3, 2) * DS                          # [NB, Z, 3, N]
    d8 = np.zeros((NB, ZP, 2, 3, N), np.float32)
    d8[:, :, 0] = Dt[:, 0:ZP]
    d8[:, 0:Z - ZP, 1] = Dt[:, ZP:Z]
    d8 = d8.astype(NPF8)

    return {
        "zc8": zc8, "wvha": wvha, "wvhb": wvhb,
        "arma": arma, "armb": armb, "wt8": wt8, "ar8": ar8, "d8": d8,
    }


def kernel(pose_body, trans, betas, A, v_template, shapedirs, posedirs,
           lbs_weights):
    if "nc" not in _CACHED:
        _CACHED["nc"] = _build_nc()
    nc = _CACHED["nc"]

    args = (pose_body, trans, betas, A, v_template, shapedirs, posedirs,
            lbs_weights)
    args = tuple(np.asarray(a, dtype=np.float32) for a in args)
    in_maps = [_prep_core(c, *args) for c in range(NCORES)]

    res = bass_utils.run_bass_kernel_spmd(nc, in_maps,
                                          core_ids=list(range(NCORES)))

    # out [NB, NCH, 128, 3*T] per core -> (B, T, N, 3)
    full = np.stack(
        [res.results[c]["out"].astype(np.float32) for c in range(NCORES)]
    )
    full = full.reshape(B, NCH, 128, 3, T).transpose(0, 4, 1, 2, 3)
    return np.ascontiguousarray(full.reshape(B, T, N, 3).astype(np.float32))


# revision 5
# speedup vs baseline: 1.0204x; 1.0080x over previous
"""Trainium2 Bass kernel for MinimalLBS (B=32, T=128, N=2048, J=52, Jb=21, L=16).

Data-parallel over B across 8 NeuronCores (4 samples per core).

Device math per sample (per 128-vertex chunk, t free):
  MAIN (exact, bf16 on PE): fold v_template+homogeneous into the stationary:
     M[n,(i,t)] = sum_{(j,k)} wvh[(j,k),n] * arm[(j,k),(i,t)]
     where wvh[(j,k),n] = wt[k,n]*vth[n,j]  (host-prepped, K=212),
     arm[(j,k),(i,t)] = A[t,k,i,j] (+ translation row at (3,J)).
  CORRECTION (fp8 DoubleRow): dv[n,j,t] (pose+shape offsets, K=206 dual-tile
     over 103 partitions) and ts8[n,(j,i,t)] (K=52 dual-tile) on PE; pm8 =
     ts8*dv on DVE (fp8 out, x32 scale in D8); j-reduction via fp8 identity
     matmuls (ident/32) accumulating straight into the M PSUM bank.

Steady state is DVE-bound: one 1325ns pm8 tensor_tensor per chunk,
back-to-back.  To keep that train unstalled, dv/dvs for chunk g+1 are
computed one window EARLY (software pipeline depth 2 on the correction
inputs), so pm8(g) only ever waits on ts8(g), which lands ~400ns into the
window.  Rodrigues pose features + betas are folded on the HOST into zc8;
PE is pre-warmed with dummy matmuls; gating DMAs are split so chunk 0's
inputs (zc8[0], first 6 n-chunks of d8[0], wt8[0], ar8[0]) land first.
"""

import sys

sys.path.insert(0, "/opt/trn_rl_repo")

import ml_dtypes
import numpy as np

import concourse.bacc as bacc
import concourse.mybir as mybir
import concourse.tile as tile
from concourse import bass_utils, masks

F32 = mybir.dt.float32
BF16 = mybir.dt.bfloat16
FP8 = mybir.dt.float8e4
NPBF16 = ml_dtypes.bfloat16
NPF8 = ml_dtypes.float8_e4m3

B, T, N, JB, J, L = 32, 128, 2048, 21, 52, 16
NCORES = 8
NB = B // NCORES          # samples per core
PF = JB * 9               # 189 pose-feature dims
Z = PF + L                # 205 combined correction coeffs
ZP = 103                  # dual-pair partitions for the z contraction (206>=205)
KA = (J + 1) * 4          # 212 (j,k) rows for the main matmul
NCH = N // 128            # n-chunks per sample
DS = 32.0                 # fp8 scale baked into D8, undone by ident/DS
D8SPLIT = 6 * 128         # first d8[0] DMA piece covers 6 n-chunks

_CACHED = {}


def _build_nc():
    nc = bacc.Bacc("TRN2", target_bir_lowering=False, debug=False)

    zc8_d = nc.dram_tensor("zc8", [ZP, 2, NB, T], FP8, kind="ExternalInput")
    wvha_d = nc.dram_tensor("wvha", [128, NB, N], BF16, kind="ExternalInput")
    wvhb_d = nc.dram_tensor("wvhb", [KA - 128, NB, N], BF16, kind="ExternalInput")
    arma_d = nc.dram_tensor("arma", [128, NB, 3 * T], BF16, kind="ExternalInput")
    armb_d = nc.dram_tensor("armb", [KA - 128, NB, 3 * T], BF16,
                            kind="ExternalInput")
    wt8_d = nc.dram_tensor("wt8", [26, 2, NB, N], FP8, kind="ExternalInput")
    ar8_d = nc.dram_tensor("ar8", [26, 2, NB, 3, 3, T], FP8, kind="ExternalInput")
    d8_d = nc.dram_tensor("d8", [NB, ZP, 2, 3, N], FP8, kind="ExternalInput")
    out_d = nc.dram_tensor("out", [NB, NCH, 128, 3 * T], BF16,
                           kind="ExternalOutput")

    with tile.TileContext(nc) as tc:
        with (
            tc.tile_pool(name="const", bufs=1) as p_const,
            tc.tile_pool(name="glob", bufs=1) as p_glob,
            tc.tile_pool(name="samp", bufs=2) as p_samp,
            tc.tile_pool(name="work", bufs=3) as p_work,
            tc.tile_pool(name="psm", bufs=2, space="PSUM") as ps_m,
            tc.tile_pool(name="psts", bufs=1, space="PSUM") as ps_ts,
        ):

            # ---- gating DMAs for chunk 0, smallest-first so the pm8 train
            # starts ASAP: zc8[0], first 6 n-chunks of d8[0], wt8[0], ar8[0].
            zc8 = p_glob.tile([ZP, 2, NB, T], FP8)
            nc.sync.dma_start(zc8[:, :, 0], zc8_d[:, :, 0])
            d8_t = [p_glob.tile([ZP, 2, 3, N], FP8, tag=f"d8_{i}",
                                name=f"d8_{i}") for i in range(NB)]
            nc.sync.dma_start(d8_t[0][:, :, :, 0:D8SPLIT],
                              d8_d[0, :, :, :, 0:D8SPLIT])
            wt8 = p_glob.tile([26, 2, NB, N], FP8)
            nc.sync.dma_start(wt8[:, :, 0], wt8_d[:, :, 0])
            ar8 = p_glob.tile([26, 2, NB, 3, 3, T], FP8)
            nc.sync.dma_start(ar8[:, :, 0], ar8_d[:, :, 0])

            # ---- main-path inputs for sample 0 (first halves first: mains(0)
            # only needs n-columns of the current chunk).
            wvha_t = [p_glob.tile([128, N], BF16, tag=f"wvha_{i}",
                                  name=f"wvha_{i}") for i in range(NB)]
            wvhb_t = [p_glob.tile([KA - 128, N], BF16, tag=f"wvhb_{i}",
                                  name=f"wvhb_{i}") for i in range(NB)]
            arma_t = [p_glob.tile([128, 3 * T], BF16, tag=f"arma_{i}",
                                  name=f"arma_{i}") for i in range(NB)]
            armb_t = [p_glob.tile([KA - 128, 3 * T], BF16, tag=f"armb_{i}",
                                  name=f"armb_{i}") for i in range(NB)]
            nc.sync.dma_start(arma_t[0][:], arma_d[:, 0])
            nc.sync.dma_start(armb_t[0][:], armb_d[:, 0])
            NH = N // 2
            nc.sync.dma_start(wvha_t[0][:, 0:NH], wvha_d[:, 0, 0:NH])
            nc.sync.dma_start(wvhb_t[0][:, 0:NH], wvhb_d[:, 0, 0:NH])
            nc.sync.dma_start(d8_t[0][:, :, :, D8SPLIT:N],
                              d8_d[0, :, :, :, D8SPLIT:N])
            nc.sync.dma_start(wvha_t[0][:, NH:N], wvha_d[:, 0, NH:N])
            nc.sync.dma_start(wvhb_t[0][:, NH:N], wvhb_d[:, 0, NH:N])
            # remaining small correction inputs for samples 1..3
            nc.sync.dma_start(zc8[:, :, 1:NB], zc8_d[:, :, 1:NB])
            nc.sync.dma_start(wt8[:, :, 1:NB], wt8_d[:, :, 1:NB])
            nc.sync.dma_start(ar8[:, :, 1:NB], ar8_d[:, :, 1:NB])

            def main_dmas(nb):
                nc.sync.dma_start(wvha_t[nb][:], wvha_d[:, nb])
                nc.sync.dma_start(arma_t[nb][:], arma_d[:, nb])
                nc.sync.dma_start(wvhb_t[nb][:], wvhb_d[:, nb])
                nc.sync.dma_start(armb_t[nb][:], armb_d[:, nb])

            identb = p_const.tile([128, 128], BF16)
            masks.make_identity(nc, identb[:])
            ident8 = p_const.tile([128, 128], FP8)
            nc.scalar.activation(ident8[:], identb[:],
                                 mybir.ActivationFunctionType.Identity,
                                 scale=1.0 / DS)
            ident8dr = p_const.tile([128, 2, 128], FP8)
            nc.scalar.copy(ident8dr[:, 0, :], ident8[:])
            nc.scalar.copy(ident8dr[:, 1, :], ident8[:])

            # ---- PE pre-warm: dummy DR matmuls (junk into the first pM pool
            # buffer, overwritten by chunk 0's start=True mains) so the 3us
            # pstate ramp completes while the gating DMAs stream in.
            pm_warm = ps_m.tile([128, 3 * T], F32, tag="M")
            for _ in range(50):
                nc.tensor.matmul(pm_warm[:, 0:128], ident8dr[:], ident8dr[:],
                                 start=True, stop=True,
                                 perf_mode=mybir.MatmulPerfMode.DoubleRow)

            # cb [128, 3(j), 512] f32 x2: cols 0:384 = ts8 slab (i,t), cols
            # 384:512 = dv (the otherwise-wasted bank pad).  Explicit ping-pong
            # (6 PSUM banks); dv for chunk g+1 is computed one window early.
            cb_t = [ps_ts.tile([128, 3, 512], F32, name=f"cb{i}")
                    for i in range(2)]
            dvs_t = [None, None, None]  # ring of 3, indexed g % 3

            def dv_chunk(g):
                nb, nch = divmod(g, NCH)
                nsl = slice(nch * 128, nch * 128 + 128)
                cb = cb_t[g % 2]
                with tc.high_priority():
                    for j in range(3):
                        nc.tensor.matmul(
                            cb[:, j, 384:512], d8_t[nb][:, :, j, nsl],
                            zc8[:, :, nb], start=True, stop=True,
                            perf_mode=mybir.MatmulPerfMode.DoubleRow,
                        )
                dvs = p_work.tile([128, 3, T], BF16, tag="dvs")
                with tc.high_priority():
                    nc.scalar.copy(dvs[:], cb[:, :, 384:512])
                dvs_t[g % 3] = dvs

            outacc_t = [None] * NB
            outacc_t[0] = p_samp.tile([128, NCH, 3 * T], BF16,
                                      tag="outacc", name="outacc")

            dv_chunk(0)  # prologue of the dv software pipeline

            NTOT = NB * NCH
            OUTB = 4  # chunks per output DMA burst
            pend = []  # staged chunks: pm8 -> (main+reduce) -> evac
            for gi in range(NTOT + 3):
                if gi < NTOT:
                    nb, nch = divmod(gi, NCH)
                    if nch == 2 and nb + 1 < NB:
                        nc.sync.dma_start(d8_t[nb + 1][:], d8_d[nb + 1])
                    if nch == 10 and nb + 1 < NB:
                        main_dmas(nb + 1)
                        outacc_t[nb + 1] = p_samp.tile(
                            [128, NCH, 3 * T], BF16, tag="outacc",
                            name="outacc")
                    nsl = slice(nch * 128, nch * 128 + 128)
                    cb = cb_t[gi % 2]
                    with tc.high_priority():
                        for j in range(3):
                            nc.tensor.matmul(
                                cb[:, j, 0:384], wt8[:, :, nb, nsl],
                                ar8[:, :, nb, j].rearrange(
                                    "k u i t -> k u (i t)"),
                                start=True, stop=True,
                                perf_mode=mybir.MatmulPerfMode.DoubleRow,
                            )
                    if gi + 1 < NTOT:
                        dv_chunk(gi + 1)

                if pend and pend[-1][4] == 0:
                    # main + j-reduce of chunk gi-1 (PE: after chunk gi's cb
                    # matmuls so cb(gi) is ready long before pm(gi) needs it)
                    _, ppm8, pnb, pnch, _st = pend[-1]
                    pM = ps_m.tile([128, 3 * T], F32, tag="M")
                    pnsl = slice(pnch * 128, pnch * 128 + 128)
                    nc.tensor.matmul(pM[:], wvha_t[pnb][:, pnsl],
                                     arma_t[pnb][:], start=True, stop=False)
                    nc.tensor.matmul(pM[:], wvhb_t[pnb][:, pnsl],
                                     armb_t[pnb][:], start=False, stop=False)
                    nc.tensor.matmul(
                        pM[:], ident8dr[:],
                        ppm8[:, 0:2].rearrange("n j i t -> n j (i t)"),
                        start=False, stop=False,
                        perf_mode=mybir.MatmulPerfMode.DoubleRow,
                        skip_group_check=True,
                    )
                    nc.tensor.matmul(
                        pM[:], ident8[:],
                        ppm8[:, 2].rearrange("n i t -> n (i t)"),
                        start=False, stop=True, skip_group_check=True,
                    )
                    pend[-1] = [gi - 1, pM, pnb, pnch, 1]

                if pend and pend[0][4] == 1 and (gi - pend[0][0] >= 2
                                                 or gi >= NTOT):
                    # evac of chunk gi-2 (ACT: two behind, so the evac's
                    # reduce-wait never delays the next dvs in ACT's queue)
                    _, pM2, pnb2, pnch2, _st = pend.pop(0)
                    nc.scalar.copy(outacc_t[pnb2][:, pnch2, :], pM2[:])
                    ob = 2 if pnb2 == NB - 1 and pnch2 >= NCH - 4 else OUTB
                    if pnch2 % ob == ob - 1:
                        c0 = pnch2 - (ob - 1)
                        nc.sync.dma_start(
                            out_d[pnb2, c0:pnch2 + 1].rearrange(
                                "c p f -> p c f"),
                            outacc_t[pnb2][:, c0:pnch2 + 1],
                        )

                if gi < NTOT:
                    # pm8 [128, 3(j), 3(i), T] fp8 = ts8 * dvs (DVE).  dvs(gi)
                    # was evac'd one window ago, so this only waits on ts8(gi).
                    pm8 = p_work.tile([128, 3, 3, T], FP8, tag="pm8")
                    nc.vector.tensor_tensor(
                        pm8[:],
                        cb[:, :, 0:384].rearrange(
                            "n j (i t) -> n j i t", i=3),
                        dvs_t[gi % 3][:].unsqueeze(2).broadcast_to(
                            (128, 3, 3, T)),
                        mybir.AluOpType.mult,
                    )
                    pend.append([gi, pm8, nb, nch, 0])

    nc.compile()
    return nc


def _rodrigues_feat(pose):
    # pose: [NB, T, JB, 3] float32 -> (R - I) flattened [NB, T, PF]
    aa = pose.astype(np.float32)
    angle = np.sqrt((aa * aa).sum(-1, keepdims=True))            # [NB,T,JB,1]
    axis = aa / np.maximum(angle, 1e-8)
    x, y, z = axis[..., 0], axis[..., 1], axis[..., 2]
    s = np.sin(angle[..., 0])[..., None, None]
    c = np.cos(angle[..., 0])[..., None, None]
    zero = np.zeros_like(x)
    K = np.stack([
        np.stack([zero, -z, y], axis=-1),
        np.stack([z, zero, -x], axis=-1),
        np.stack([-y, x, zero], axis=-1),
    ], axis=-2)
    outer = axis[..., :, None] * axis[..., None, :]
    I = np.eye(3, dtype=np.float32)
    R = c * I + s * K + (1.0 - c) * outer
    return (R - I).reshape(aa.shape[0], aa.shape[1], PF)


def _prep_core(c, pose_body, trans, betas, A, v_template, shapedirs, posedirs,
               lbs_weights):
    bs = slice(NB * c, NB * (c + 1))

    # zc = [pose_feature | betas] per (sample, t), packed into fp8 dual-pairs
    # (z, z+103); the phantom row z=205 is the zero at (102, u=1).
    pf = _rodrigues_feat(pose_body[bs].reshape(NB, T, JB, 3))    # [NB,T,PF]
    zc = np.concatenate([pf, betas[bs]], axis=2)                 # [NB,T,Z]
    zcT = np.ascontiguousarray(zc.transpose(2, 0, 1))            # [Z,NB,T]
    zc8 = np.zeros((ZP, 2, NB, T), np.float32)
    zc8[:, 0] = zcT[0:ZP]
    zc8[0:Z - ZP, 1] = zcT[ZP:Z]
    zc8 = zc8.astype(NPF8)

    wt = np.concatenate(
        [lbs_weights[bs].transpose(0, 2, 1),
         np.ones((NB, 1, N), np.float32)], axis=1)             # [NB, 53, N]
    vth = np.concatenate(
        [v_template[bs], np.ones((NB, N, 1), np.float32)], axis=2)  # [NB,N,4]
    wvh = (vth.transpose(0, 2, 1)[:, :, None, :] * wt[:, None, :, :]
           ).reshape(NB, KA, N)                                # [NB,(j,k),N]
    wvha = np.ascontiguousarray(wvh[:, 0:128].transpose(1, 0, 2)).astype(NPBF16)
    wvhb = np.ascontiguousarray(wvh[:, 128:KA].transpose(1, 0, 2)).astype(NPBF16)

    arm = np.zeros((NB, 4, J + 1, 3, T), np.float32)
    arm[:, :, :J] = A[bs, :, :, 0:3, :].transpose(0, 4, 2, 3, 1)  # [nb,j,k,i,t]
    arm[:, 3, J] = trans[bs].transpose(0, 2, 1)                   # [nb,i,t]
    arm = arm.reshape(NB, KA, 3 * T)
    arma = np.ascontiguousarray(arm[:, 0:128].transpose(1, 0, 2)).astype(NPBF16)
    armb = np.ascontiguousarray(arm[:, 128:KA].transpose(1, 0, 2)).astype(NPBF16)

    wt8 = np.empty((26, 2, NB, N), np.float32)
    wt8[:, 0] = wt[:, 0:26].transpose(1, 0, 2)
    wt8[:, 1] = wt[:, 26:52].transpose(1, 0, 2)
    wt8 = wt8.astype(NPF8)

    ar8f = A[bs, :, :, 0:3, 0:3].transpose(0, 2, 4, 3, 1)      # [nb,k,j,i,t]
    ar8 = np.empty((26, 2, NB, 3, 3, T), np.float32)
    ar8[:, 0] = ar8f[:, 0:26].transpose(1, 0, 2, 3, 4)
    ar8[:, 1] = ar8f[:, 26:52].transpose(1, 0, 2, 3, 4)
    ar8 = ar8.astype(NPF8)

    D = np.concatenate([
        posedirs[bs].reshape(NB, PF, N, 3),
        shapedirs[bs].transpose(0, 3, 1, 2),                   # [NB, L, N, 3]
    ], axis=1)                                                 # [NB, Z, N, 3]
    Dt = D.transpose(0, 1, 3, 2) * DS                          # [NB, Z, 3, N]
    d8 = np.zeros((NB, ZP, 2, 3, N), np.float32)
    d8[:, :, 0] = Dt[:, 0:ZP]
    d8[:, 0:Z - ZP, 1] = Dt[:, ZP:Z]
    d8 = d8.astype(NPF8)

    return {
        "zc8": zc8, "wvha": wvha, "wvhb": wvhb,
        "arma": arma, "armb": armb, "wt8": wt8, "ar8": ar8, "d8": d8,
    }


def kernel(pose_body, trans, betas, A, v_template, shapedirs, posedirs,
           lbs_weights):
    if "nc" not in _CACHED:
        _CACHED["nc"] = _build_nc()
    nc = _CACHED["nc"]

    args = (pose_body, trans, betas, A, v_template, shapedirs, posedirs,
            lbs_weights)
    args = tuple(np.asarray(a, dtype=np.float32) for a in args)
    in_maps = [_prep_core(c, *args) for c in range(NCORES)]

    res = bass_utils.run_bass_kernel_spmd(nc, in_maps,
                                          core_ids=list(range(NCORES)))

    # out [NB, NCH, 128, 3*T] per core -> (B, T, N, 3)
    full = np.stack(
        [res.results[c]["out"].astype(np.float32) for c in range(NCORES)]
    )
    full = full.reshape(B, NCH, 128, 3, T).transpose(0, 4, 1, 2, 3)
    return np.ascontiguousarray(full.reshape(B, T, N, 3).astype(np.float32))


# revision 6
# speedup vs baseline: 1.1083x; 1.0862x over previous
"""Trainium2 Bass kernel for MinimalLBS (B=32, T=128, N=2048, J=52, Jb=21, L=16).

Data-parallel over B across 8 NeuronCores (4 samples per core).

Device math per sample (per 128-vertex chunk, t free):
  MAIN (exact, bf16 on PE): fold v_template+homogeneous into the stationary:
     M[n,(i,t)] = sum_{(j,k)} wvh[(j,k),n] * arm[(j,k),(i,t)]
     where wvh[(j,k),n] = wt[k,n]*vth[n,j]  (host-prepped, K=212),
     arm[(j,k),(i,t)] = A[t,k,i,j] (+ translation row at (3,J)).
  CORRECTION (fp8 DoubleRow): dv[n,j,t] (pose+shape offsets, K=206 dual-tile
     over 103 partitions) and ts8[n,(j,i,t)] (K=52 dual-tile) on PE; pm8 =
     ts8*dv on DVE (fp8 out, x32 scale in D8); j-reduction via fp8 identity
     matmuls (ident/32) accumulating straight into the M PSUM bank.

Steady state is DVE-bound: one 1325ns pm8 tensor_tensor per chunk,
back-to-back.  To keep that train unstalled, dv/dvs for chunk g+1 are
computed one window EARLY (software pipeline depth 2 on the correction
inputs), so pm8(g) only ever waits on ts8(g), which lands ~400ns into the
window.  Rodrigues pose features + betas are folded on the HOST into zc8;
PE is pre-warmed with dummy matmuls; gating DMAs are split so chunk 0's
inputs (zc8[0], first 6 n-chunks of d8[0], wt8[0], ar8[0]) land first.
"""

import sys

sys.path.insert(0, "/opt/trn_rl_repo")

import ml_dtypes
import numpy as np

import concourse.bacc as bacc
import concourse.mybir as mybir
import concourse.tile as tile
from concourse import bass_utils, masks

F32 = mybir.dt.float32
BF16 = mybir.dt.bfloat16
FP8 = mybir.dt.float8e4
NPBF16 = ml_dtypes.bfloat16
NPF8 = ml_dtypes.float8_e4m3

B, T, N, JB, J, L = 32, 128, 2048, 21, 52, 16
NCORES = 8
NB = B // NCORES          # samples per core
PF = JB * 9               # 189 pose-feature dims
Z = PF + L                # 205 combined correction coeffs
ZP = 103                  # dual-pair partitions for the z contraction (206>=205)
KA = (J + 1) * 4          # 212 (j,k) rows for the main matmul
NCH = N // 128            # n-chunks per sample
DS = 32.0                 # fp8 scale baked into D8, undone by ident/DS
D8SPLIT = 6 * 128         # first d8[0] DMA piece covers 6 n-chunks

_CACHED = {}


def _build_nc():
    nc = bacc.Bacc("TRN2", target_bir_lowering=False, debug=False)

    zc8_d = nc.dram_tensor("zc8", [ZP, 2, NB, T], FP8, kind="ExternalInput")
    wvha_d = nc.dram_tensor("wvha", [128, NB, N], BF16, kind="ExternalInput")
    wvhb_d = nc.dram_tensor("wvhb", [KA - 128, NB, N], BF16, kind="ExternalInput")
    arma_d = nc.dram_tensor("arma", [128, NB, 3 * T], BF16, kind="ExternalInput")
    armb_d = nc.dram_tensor("armb", [KA - 128, NB, 3 * T], BF16,
                            kind="ExternalInput")
    wt8_d = nc.dram_tensor("wt8", [26, 2, NB, N], FP8, kind="ExternalInput")
    ar8_d = nc.dram_tensor("ar8", [26, 2, NB, 3, 3, T], FP8, kind="ExternalInput")
    d8_d = nc.dram_tensor("d8", [NB, ZP, 2, 3, N], FP8, kind="ExternalInput")
    out_d = nc.dram_tensor("out", [NB, NCH, 128, 3 * T], BF16,
                           kind="ExternalOutput")

    with tile.TileContext(nc) as tc:
        with (
            tc.tile_pool(name="const", bufs=1) as p_const,
            tc.tile_pool(name="glob", bufs=1) as p_glob,
            tc.tile_pool(name="samp", bufs=2) as p_samp,
            tc.tile_pool(name="work", bufs=3) as p_work,
            tc.tile_pool(name="psm", bufs=2, space="PSUM") as ps_m,
            tc.tile_pool(name="psts", bufs=1, space="PSUM") as ps_ts,
        ):

            # ---- gating DMAs for chunk 0, smallest-first so the pm8 train
            # starts ASAP: zc8[0], first 6 n-chunks of d8[0], wt8[0], ar8[0].
            zc8 = p_glob.tile([ZP, 2, NB, T], FP8)
            nc.sync.dma_start(zc8[:, :, 0], zc8_d[:, :, 0])
            d8_t = [p_glob.tile([ZP, 2, 3, N], FP8, tag=f"d8_{i}",
                                name=f"d8_{i}") for i in range(NB)]
            nc.sync.dma_start(d8_t[0][:, :, :, 0:D8SPLIT],
                              d8_d[0, :, :, :, 0:D8SPLIT])
            wt8 = p_glob.tile([26, 2, NB, N], FP8)
            nc.sync.dma_start(wt8[:, :, 0], wt8_d[:, :, 0])
            ar8 = p_glob.tile([26, 2, NB, 3, 3, T], FP8)
            nc.sync.dma_start(ar8[:, :, 0], ar8_d[:, :, 0])

            # ---- main-path inputs for sample 0 (first halves first: mains(0)
            # only needs n-columns of the current chunk).
            wvha_t = [p_glob.tile([128, N], BF16, tag=f"wvha_{i}",
                                  name=f"wvha_{i}") for i in range(NB)]
            wvhb_t = [p_glob.tile([KA - 128, N], BF16, tag=f"wvhb_{i}",
                                  name=f"wvhb_{i}") for i in range(NB)]
            arma_t = [p_glob.tile([128, 3 * T], BF16, tag=f"arma_{i}",
                                  name=f"arma_{i}") for i in range(NB)]
            armb_t = [p_glob.tile([KA - 128, 3 * T], BF16, tag=f"armb_{i}",
                                  name=f"armb_{i}") for i in range(NB)]
            nc.sync.dma_start(arma_t[0][:], arma_d[:, 0])
            nc.sync.dma_start(armb_t[0][:], armb_d[:, 0])
            NH = N // 2
            nc.sync.dma_start(wvha_t[0][:, 0:NH], wvha_d[:, 0, 0:NH])
            nc.sync.dma_start(wvhb_t[0][:, 0:NH], wvhb_d[:, 0, 0:NH])
            nc.sync.dma_start(d8_t[0][:, :, :, D8SPLIT:N],
                              d8_d[0, :, :, :, D8SPLIT:N])
            nc.sync.dma_start(wvha_t[0][:, NH:N], wvha_d[:, 0, NH:N])
            nc.sync.dma_start(wvhb_t[0][:, NH:N], wvhb_d[:, 0, NH:N])
            # remaining small correction inputs for samples 1..3
            nc.sync.dma_start(zc8[:, :, 1:NB], zc8_d[:, :, 1:NB])
            nc.sync.dma_start(wt8[:, :, 1:NB], wt8_d[:, :, 1:NB])
            nc.sync.dma_start(ar8[:, :, 1:NB], ar8_d[:, :, 1:NB])

            def main_dmas(nb):
                nc.sync.dma_start(wvha_t[nb][:], wvha_d[:, nb])
                nc.sync.dma_start(arma_t[nb][:], arma_d[:, nb])
                nc.sync.dma_start(wvhb_t[nb][:], wvhb_d[:, nb])
                nc.sync.dma_start(armb_t[nb][:], armb_d[:, nb])

            identb = p_const.tile([128, 128], BF16)
            nc.gpsimd.memset(identb[:], 0.0)
            nc.gpsimd.affine_select(
                out=identb[:], in_=identb[:],
                compare_op=mybir.AluOpType.not_equal,
                fill=1.0 / DS, base=0, pattern=[[-1, 128]],
                channel_multiplier=1)
            ident8 = p_const.tile([128, 128], FP8)
            nc.scalar.copy(ident8[:], identb[:])
            ident8dr = p_const.tile([128, 2, 128], FP8)
            nc.scalar.copy(ident8dr[:, 0, :], ident8[:])
            nc.scalar.copy(ident8dr[:, 1, :], ident8[:])

            # ---- PE pre-warm: dummy DR matmuls (junk into the first pM pool
            # buffer, overwritten by chunk 0's start=True mains) so the 3us
            # pstate ramp completes while the gating DMAs stream in.
            pm_warm = ps_m.tile([128, 3 * T], F32, tag="M")
            for _ in range(50):
                nc.tensor.matmul(pm_warm[:, 0:128], ident8dr[:], ident8dr[:],
                                 start=True, stop=True,
                                 perf_mode=mybir.MatmulPerfMode.DoubleRow)

            # cb [128, 3(j), 512] f32 x2: cols 0:384 = ts8(k) slab (i,t) for
            # k%2 == b; cols 384:512 (the otherwise-wasted bank pads) hold
            # dv(k+1) -- the CROSSED buffer.  Each tile thus gets one write
            # burst (ts8(k+1) then dv(k+2), back-to-back on PE at the top of
            # window k) followed by its readers (pm8(k+1) next window,
            # dvs(k+2) this window), so with the tile-granular dependency
            # tracker no write ever waits behind a long read: the pm8 train
            # runs back-to-back at 1325ns.
            cb_t = [ps_ts.tile([128, 3, 512], F32, name=f"cb{i}")
                    for i in range(2)]
            dvs_t = [None, None, None]  # ring of 3, indexed g % 3

            def ts8_chunk(g):
                nb, nch = divmod(g, NCH)
                nsl = slice(nch * 128, nch * 128 + 128)
                cb = cb_t[g % 2]
                with tc.high_priority():
                    for j in range(3):
                        nc.tensor.matmul(
                            cb[:, j, 0:384], wt8[:, :, nb, nsl],
                            ar8[:, :, nb, j].rearrange("k u i t -> k u (i t)"),
                            start=True, stop=True,
                            perf_mode=mybir.MatmulPerfMode.DoubleRow,
                        )

            def dv_chunk(g):
                nb, nch = divmod(g, NCH)
                nsl = slice(nch * 128, nch * 128 + 128)
                cb = cb_t[(g + 1) % 2]      # crossed: pads of the other buffer
                with tc.high_priority():
                    for j in range(3):
                        nc.tensor.matmul(
                            cb[:, j, 384:512], d8_t[nb][:, :, j, nsl],
                            zc8[:, :, nb], start=True, stop=True,
                            perf_mode=mybir.MatmulPerfMode.DoubleRow,
                        )
                dvs = p_work.tile([128, 3, T], BF16, tag="dvs")
                with tc.high_priority():
                    nc.scalar.copy(dvs[:], cb[:, :, 384:512])
                dvs_t[g % 3] = dvs

            outacc_t = [None] * NB
            outacc_t[0] = p_samp.tile([128, NCH, 3 * T], BF16,
                                      tag="outacc", name="outacc")

            # software-pipeline prologue
            dv_chunk(0)
            ts8_chunk(0)
            dv_chunk(1)

            NTOT = NB * NCH
            OUTB = 4  # chunks per output DMA burst
            pend = []  # staged chunks: pm8 -> (main+reduce) -> evac
            for gi in range(NTOT + 3):
                if gi < NTOT:
                    nb, nch = divmod(gi, NCH)
                    if nch == 2 and nb + 1 < NB:
                        nc.sync.dma_start(d8_t[nb + 1][:], d8_d[nb + 1])
                    if nch == 10 and nb + 1 < NB:
                        main_dmas(nb + 1)
                        outacc_t[nb + 1] = p_samp.tile(
                            [128, NCH, 3 * T], BF16, tag="outacc",
                            name="outacc")

                if pend and pend[-1][4] == 0:
                    # main + j-reduce of chunk gi-1 (PE, top of window gi)
                    _, ppm8, pnb, pnch, _st = pend[-1]
                    pM = ps_m.tile([128, 3 * T], F32, tag="M")
                    pnsl = slice(pnch * 128, pnch * 128 + 128)
                    nc.tensor.matmul(pM[:], wvha_t[pnb][:, pnsl],
                                     arma_t[pnb][:], start=True, stop=False)
                    nc.tensor.matmul(pM[:], wvhb_t[pnb][:, pnsl],
                                     armb_t[pnb][:], start=False, stop=False)
                    nc.tensor.matmul(
                        pM[:], ident8dr[:],
                        ppm8[:, 0:2].rearrange("n j i t -> n j (i t)"),
                        start=False, stop=False,
                        perf_mode=mybir.MatmulPerfMode.DoubleRow,
                        skip_group_check=True,
                    )
                    nc.tensor.matmul(
                        pM[:], ident8[:],
                        ppm8[:, 2].rearrange("n i t -> n (i t)"),
                        start=False, stop=True, skip_group_check=True,
                    )
                    pend[-1] = [gi - 1, pM, pnb, pnch, 1]

                if pend and pend[0][4] == 1 and (gi - pend[0][0] >= 2
                                                 or gi >= NTOT):
                    # evac of chunk gi-2 (ACT: two behind, so the evac's
                    # reduce-wait never delays the next dvs in ACT's queue)
                    _, pM2, pnb2, pnch2, _st = pend.pop(0)
                    nc.scalar.copy(outacc_t[pnb2][:, pnch2, :], pM2[:])
                    ob = 2 if pnb2 == NB - 1 and pnch2 >= NCH - 4 else OUTB
                    if pnch2 % ob == ob - 1:
                        c0 = pnch2 - (ob - 1)
                        nc.sync.dma_start(
                            out_d[pnb2, c0:pnch2 + 1].rearrange(
                                "c p f -> p c f"),
                            outacc_t[pnb2][:, c0:pnch2 + 1],
                        )

                if gi < NTOT:
                    # pm8 [128, 3(j), 3(i), T] fp8 = ts8 * dvs (DVE); both
                    # inputs were produced a window ago -- never stalls.
                    pm8 = p_work.tile([128, 3, 3, T], FP8, tag="pm8")
                    nc.vector.tensor_tensor(
                        pm8[:],
                        cb_t[gi % 2][:, :, 0:384].rearrange(
                            "n j (i t) -> n j i t", i=3),
                        dvs_t[gi % 3][:].unsqueeze(2).broadcast_to(
                            (128, 3, 3, T)),
                        mybir.AluOpType.mult,
                    )
                    pend.append([gi, pm8, nb, nch, 0])
                    if gi + 1 < NTOT:
                        ts8_chunk(gi + 1)
                    if gi + 2 < NTOT:
                        dv_chunk(gi + 2)

    nc.compile()
    return nc


def _rodrigues_feat(pose):
    # pose: [NB, T, JB, 3] float32 -> (R - I) flattened [NB, T, PF]
    aa = pose.astype(np.float32)
    angle = np.sqrt((aa * aa).sum(-1, keepdims=True))            # [NB,T,JB,1]
    axis = aa / np.maximum(angle, 1e-8)
    x, y, z = axis[..., 0], axis[..., 1], axis[..., 2]
    s = np.sin(angle[..., 0])[..., None, None]
    c = np.cos(angle[..., 0])[..., None, None]
    zero = np.zeros_like(x)
    K = np.stack([
        np.stack([zero, -z, y], axis=-1),
        np.stack([z, zero, -x], axis=-1),
        np.stack([-y, x, zero], axis=-1),
    ], axis=-2)
    outer = axis[..., :, None] * axis[..., None, :]
    I = np.eye(3, dtype=np.float32)
    R = c * I + s * K + (1.0 - c) * outer
    return (R - I).reshape(aa.shape[0], aa.shape[1], PF)


def _prep_core(c, pose_body, trans, betas, A, v_template, shapedirs, posedirs,
               lbs_weights):
    bs = slice(NB * c, NB * (c + 1))

    # zc = [pose_feature | betas] per (sample, t), packed into fp8 dual-pairs
    # (z, z+103); the phantom row z=205 is the zero at (102, u=1).
    pf = _rodrigues_feat(pose_body[bs].reshape(NB, T, JB, 3))    # [NB,T,PF]
    zc = np.concatenate([pf, betas[bs]], axis=2)                 # [NB,T,Z]
    zcT = np.ascontiguousarray(zc.transpose(2, 0, 1))            # [Z,NB,T]
    zc8 = np.zeros((ZP, 2, NB, T), np.float32)
    zc8[:, 0] = zcT[0:ZP]
    zc8[0:Z - ZP, 1] = zcT[ZP:Z]
    zc8 = zc8.astype(NPF8)

    wt = np.concatenate(
        [lbs_weights[bs].transpose(0, 2, 1),
         np.ones((NB, 1, N), np.float32)], axis=1)             # [NB, 53, N]
    vth = np.concatenate(
        [v_template[bs], np.ones((NB, N, 1), np.float32)], axis=2)  # [NB,N,4]
    wvh = (vth.transpose(0, 2, 1)[:, :, None, :] * wt[:, None, :, :]
           ).reshape(NB, KA, N)                                # [NB,(j,k),N]
    wvha = np.ascontiguousarray(wvh[:, 0:128].transpose(1, 0, 2)).astype(NPBF16)
    wvhb = np.ascontiguousarray(wvh[:, 128:KA].transpose(1, 0, 2)).astype(NPBF16)

    arm = np.zeros((NB, 4, J + 1, 3, T), np.float32)
    arm[:, :, :J] = A[bs, :, :, 0:3, :].transpose(0, 4, 2, 3, 1)  # [nb,j,k,i,t]
    arm[:, 3, J] = trans[bs].transpose(0, 2, 1)                   # [nb,i,t]
    arm = arm.reshape(NB, KA, 3 * T)
    arma = np.ascontiguousarray(arm[:, 0:128].transpose(1, 0, 2)).astype(NPBF16)
    armb = np.ascontiguousarray(arm[:, 128:KA].transpose(1, 0, 2)).astype(NPBF16)

    wt8 = np.empty((26, 2, NB, N), np.float32)
    wt8[:, 0] = wt[:, 0:26].transpose(1, 0, 2)
    wt8[:, 1] = wt[:, 26:52].transpose(1, 0, 2)
    wt8 = wt8.astype(NPF8)

    ar8f = A[bs, :, :, 0:3, 0:3].transpose(0, 2, 4, 3, 1)      # [nb,k,j,i,t]
    ar8 = np.empty((26, 2, NB, 3, 3, T), np.float32)
    ar8[:, 0] = ar8f[:, 0:26].transpose(1, 0, 2, 3, 4)
    ar8[:, 1] = ar8f[:, 26:52].transpose(1, 0, 2, 3, 4)
    ar8 = ar8.astype(NPF8)

    D = np.concatenate([
        posedirs[bs].reshape(NB, PF, N, 3),
        shapedirs[bs].transpose(0, 3, 1, 2),                   # [NB, L, N, 3]
    ], axis=1)                                                 # [NB, Z, N, 3]
    Dt = D.transpose(0, 1, 3, 2) * DS                          # [NB, Z, 3, N]
    d8 = np.zeros((NB, ZP, 2, 3, N), np.float32)
    d8[:, :, 0] = Dt[:, 0:ZP]
    d8[:, 0:Z - ZP, 1] = Dt[:, ZP:Z]
    d8 = d8.astype(NPF8)

    return {
        "zc8": zc8, "wvha": wvha, "wvhb": wvhb,
        "arma": arma, "armb": armb, "wt8": wt8, "ar8": ar8, "d8": d8,
    }


def kernel(pose_body, trans, betas, A, v_template, shapedirs, posedirs,
           lbs_weights):
    if "nc" not in _CACHED:
        _CACHED["nc"] = _build_nc()
    nc = _CACHED["nc"]

    args = (pose_body, trans, betas, A, v_template, shapedirs, posedirs,
            lbs_weights)
    args = tuple(np.asarray(a, dtype=np.float32) for a in args)
    in_maps = [_prep_core(c, *args) for c in range(NCORES)]

    res = bass_utils.run_bass_kernel_spmd(nc, in_maps,
                                          core_ids=list(range(NCORES)))

    # out [NB, NCH, 128, 3*T] per core -> (B, T, N, 3)
    full = np.stack(
        [res.results[c]["out"].astype(np.float32) for c in range(NCORES)]
    )
    full = full.reshape(B, NCH, 128, 3, T).transpose(0, 4, 1, 2, 3)
    return np.ascontiguousarray(full.reshape(B, T, N, 3).astype(np.float32))


# revision 7
# speedup vs baseline: 1.1158x; 1.0068x over previous
"""Trainium2 Bass kernel for MinimalLBS (B=32, T=128, N=2048, J=52, Jb=21, L=16).

Data-parallel over B across 8 NeuronCores (4 samples per core).

Device math per sample (per 128-vertex chunk, t free):
  MAIN (exact, bf16 on PE): fold v_template+homogeneous into the stationary:
     M[n,(i,t)] = sum_{(j,k)} wvh[(j,k),n] * arm[(j,k),(i,t)]
     where wvh[(j,k),n] = wt[k,n]*vth[n,j]  (host-prepped, K=212),
     arm[(j,k),(i,t)] = A[t,k,i,j] (+ translation row at (3,J)).
  CORRECTION (fp8 DoubleRow): dv[n,j,t] (pose+shape offsets, K=206 dual-tile
     over 103 partitions) and ts8[n,(j,i,t)] (K=52 dual-tile) on PE; pm8 =
     ts8*dv on DVE (fp8 out, x32 scale in D8); j-reduction via fp8 identity
     matmuls (ident/32) accumulating straight into the M PSUM bank.

Steady state is DVE-bound: one 1325ns pm8 tensor_tensor per chunk,
back-to-back.  To keep that train unstalled, dv/dvs for chunk g+1 are
computed one window EARLY (software pipeline depth 2 on the correction
inputs), so pm8(g) only ever waits on ts8(g), which lands ~400ns into the
window.  Rodrigues pose features + betas are folded on the HOST into zc8;
PE is pre-warmed with dummy matmuls; gating DMAs are split so chunk 0's
inputs (zc8[0], first 6 n-chunks of d8[0], wt8[0], ar8[0]) land first.
"""

import sys

sys.path.insert(0, "/opt/trn_rl_repo")

import ml_dtypes
import numpy as np

import concourse.bacc as bacc
import concourse.mybir as mybir
import concourse.tile as tile
from concourse import bass_utils, masks

F32 = mybir.dt.float32
BF16 = mybir.dt.bfloat16
FP8 = mybir.dt.float8e4
NPBF16 = ml_dtypes.bfloat16
NPF8 = ml_dtypes.float8_e4m3

B, T, N, JB, J, L = 32, 128, 2048, 21, 52, 16
NCORES = 8
NB = B // NCORES          # samples per core
PF = JB * 9               # 189 pose-feature dims
Z = PF + L                # 205 combined correction coeffs
ZP = 103                  # dual-pair partitions for the z contraction (206>=205)
KA = (J + 1) * 4          # 212 (j,k) rows for the main matmul
NCH = N // 128            # n-chunks per sample
DS = 32.0                 # fp8 scale baked into D8, undone by ident/DS
D8SPLIT = 6 * 128         # first d8[0] DMA piece covers 6 n-chunks

_CACHED = {}


def _build_nc():
    nc = bacc.Bacc("TRN2", target_bir_lowering=False, debug=False)

    zc8_d = nc.dram_tensor("zc8", [ZP, 2, NB, T], FP8, kind="ExternalInput")
    wvha_d = nc.dram_tensor("wvha", [128, NB, N], BF16, kind="ExternalInput")
    wvhb_d = nc.dram_tensor("wvhb", [KA - 128, NB, N], BF16, kind="ExternalInput")
    arma_d = nc.dram_tensor("arma", [128, NB, 3 * T], BF16, kind="ExternalInput")
    armb_d = nc.dram_tensor("armb", [KA - 128, NB, 3 * T], BF16,
                            kind="ExternalInput")
    wt8_d = nc.dram_tensor("wt8", [26, 2, NB, N], FP8, kind="ExternalInput")
    ar8_d = nc.dram_tensor("ar8", [26, 2, NB, 3, 3, T], FP8, kind="ExternalInput")
    d8_d = nc.dram_tensor("d8", [NB, ZP, 2, 3, N], FP8, kind="ExternalInput")
    out_d = nc.dram_tensor("out", [NB, NCH, 128, 3 * T], BF16,
                           kind="ExternalOutput")

    with tile.TileContext(nc) as tc:
        with (
            tc.tile_pool(name="const", bufs=1) as p_const,
            tc.tile_pool(name="glob", bufs=1) as p_glob,
            tc.tile_pool(name="samp", bufs=2) as p_samp,
            tc.tile_pool(name="work", bufs=4) as p_work,
            tc.tile_pool(name="psm", bufs=2, space="PSUM") as ps_m,
            tc.tile_pool(name="psts", bufs=1, space="PSUM") as ps_ts,
        ):

            # ---- gating DMAs for chunk 0, smallest-first so the pm8 train
            # starts ASAP: zc8[0], first 6 n-chunks of d8[0], wt8[0], ar8[0].
            zc8 = p_glob.tile([ZP, 2, NB, T], FP8)
            nc.sync.dma_start(zc8[:, :, 0], zc8_d[:, :, 0])
            d8_t = [p_glob.tile([ZP, 2, 3, N], FP8, tag=f"d8_{i}",
                                name=f"d8_{i}") for i in range(NB)]
            nc.sync.dma_start(d8_t[0][:, :, :, 0:D8SPLIT],
                              d8_d[0, :, :, :, 0:D8SPLIT])
            wt8 = p_glob.tile([26, 2, NB, N], FP8)
            nc.sync.dma_start(wt8[:, :, 0], wt8_d[:, :, 0])
            ar8 = p_glob.tile([26, 2, NB, 3, 3, T], FP8)
            nc.sync.dma_start(ar8[:, :, 0], ar8_d[:, :, 0])

            # ---- main-path inputs for sample 0 (first halves first: mains(0)
            # only needs n-columns of the current chunk).
            wvha_t = [p_glob.tile([128, N], BF16, tag=f"wvha_{i}",
                                  name=f"wvha_{i}") for i in range(NB)]
            wvhb_t = [p_glob.tile([KA - 128, N], BF16, tag=f"wvhb_{i}",
                                  name=f"wvhb_{i}") for i in range(NB)]
            arma_t = [p_glob.tile([128, 3 * T], BF16, tag=f"arma_{i}",
                                  name=f"arma_{i}") for i in range(NB)]
            armb_t = [p_glob.tile([KA - 128, 3 * T], BF16, tag=f"armb_{i}",
                                  name=f"armb_{i}") for i in range(NB)]
            nc.sync.dma_start(arma_t[0][:], arma_d[:, 0])
            nc.sync.dma_start(armb_t[0][:], armb_d[:, 0])
            NH = N // 2
            nc.sync.dma_start(wvha_t[0][:, 0:NH], wvha_d[:, 0, 0:NH])
            nc.sync.dma_start(wvhb_t[0][:, 0:NH], wvhb_d[:, 0, 0:NH])
            nc.sync.dma_start(d8_t[0][:, :, :, D8SPLIT:N],
                              d8_d[0, :, :, :, D8SPLIT:N])
            nc.sync.dma_start(wvha_t[0][:, NH:N], wvha_d[:, 0, NH:N])
            nc.sync.dma_start(wvhb_t[0][:, NH:N], wvhb_d[:, 0, NH:N])
            # remaining small correction inputs for samples 1..3
            nc.sync.dma_start(zc8[:, :, 1:NB], zc8_d[:, :, 1:NB])
            nc.sync.dma_start(wt8[:, :, 1:NB], wt8_d[:, :, 1:NB])
            nc.sync.dma_start(ar8[:, :, 1:NB], ar8_d[:, :, 1:NB])

            def main_dmas(nb):
                nc.sync.dma_start(wvha_t[nb][:], wvha_d[:, nb])
                nc.sync.dma_start(arma_t[nb][:], arma_d[:, nb])
                nc.sync.dma_start(wvhb_t[nb][:], wvhb_d[:, nb])
                nc.sync.dma_start(armb_t[nb][:], armb_d[:, nb])

            identb = p_const.tile([128, 128], BF16)
            nc.gpsimd.memset(identb[:], 0.0)
            nc.gpsimd.affine_select(
                out=identb[:], in_=identb[:],
                compare_op=mybir.AluOpType.not_equal,
                fill=1.0 / DS, base=0, pattern=[[-1, 128]],
                channel_multiplier=1)
            ident8 = p_const.tile([128, 128], FP8)
            nc.scalar.copy(ident8[:], identb[:])
            ident8dr = p_const.tile([128, 2, 128], FP8)
            nc.scalar.copy(ident8dr[:, 0, :], ident8[:])
            nc.scalar.copy(ident8dr[:, 1, :], ident8[:])

            # ---- PE pre-warm: dummy DR matmuls (junk into the first pM pool
            # buffer, overwritten by chunk 0's start=True mains) so the 3us
            # pstate ramp completes while the gating DMAs stream in.
            pm_warm = ps_m.tile([128, 3 * T], F32, tag="M")
            for _ in range(50):
                nc.tensor.matmul(pm_warm[:, 0:128], ident8dr[:], ident8dr[:],
                                 start=True, stop=True,
                                 perf_mode=mybir.MatmulPerfMode.DoubleRow)

            # cb [128, 3(j), 512] f32 x2: cols 0:384 = ts8(k) slab (i,t) for
            # k%2 == b; cols 384:512 (the otherwise-wasted bank pads) hold
            # dv(k+1) -- the CROSSED buffer.  Each tile thus gets one write
            # burst (ts8(k+1) then dv(k+2), back-to-back on PE at the top of
            # window k) followed by its readers (pm8(k+1) next window,
            # dvs(k+2) this window), so with the tile-granular dependency
            # tracker no write ever waits behind a long read: the pm8 train
            # runs back-to-back at 1325ns.
            cb_t = [ps_ts.tile([128, 3, 512], F32, name=f"cb{i}")
                    for i in range(2)]
            dvs_t = [None] * 4  # ring of 4, indexed g % 4

            def ts8_chunk(g):
                nb, nch = divmod(g, NCH)
                nsl = slice(nch * 128, nch * 128 + 128)
                cb = cb_t[g % 2]
                with tc.high_priority():
                    for j in range(3):
                        nc.tensor.matmul(
                            cb[:, j, 0:384], wt8[:, :, nb, nsl],
                            ar8[:, :, nb, j].rearrange("k u i t -> k u (i t)"),
                            start=True, stop=True,
                            perf_mode=mybir.MatmulPerfMode.DoubleRow,
                        )

            def dv_chunk(g):
                nb, nch = divmod(g, NCH)
                nsl = slice(nch * 128, nch * 128 + 128)
                cb = cb_t[(g + 1) % 2]      # crossed: pads of the other buffer
                with tc.high_priority():
                    for j in range(3):
                        nc.tensor.matmul(
                            cb[:, j, 384:512], d8_t[nb][:, :, j, nsl],
                            zc8[:, :, nb], start=True, stop=True,
                            perf_mode=mybir.MatmulPerfMode.DoubleRow,
                        )
                dvs = p_work.tile([128, 3, T], BF16, tag="dvs")
                with tc.high_priority():
                    nc.scalar.copy(dvs[:], cb[:, :, 384:512])
                dvs_t[g % 4] = dvs

            outacc_t = [None] * NB
            outacc_t[0] = p_samp.tile([128, NCH, 3 * T], BF16,
                                      tag="outacc", name="outacc")

            # software-pipeline prologue (depth 3)
            dv_chunk(0)
            ts8_chunk(0)
            dv_chunk(1)
            ts8_chunk(1)
            dv_chunk(2)

            NTOT = NB * NCH
            OUTB = 4  # chunks per output DMA burst
            pend = []  # staged chunks: pm8 -> (main+reduce) -> evac
            for gi in range(NTOT + 3):
                if gi < NTOT:
                    nb, nch = divmod(gi, NCH)
                    if nch == 2 and nb + 1 < NB:
                        nc.sync.dma_start(d8_t[nb + 1][:], d8_d[nb + 1])
                    if nch == 10 and nb + 1 < NB:
                        main_dmas(nb + 1)
                        outacc_t[nb + 1] = p_samp.tile(
                            [128, NCH, 3 * T], BF16, tag="outacc",
                            name="outacc")

                if pend and pend[-1][4] == 0:
                    # main + j-reduce of chunk gi-1 (PE, top of window gi)
                    _, ppm8, pnb, pnch, _st = pend[-1]
                    pM = ps_m.tile([128, 3 * T], F32, tag="M")
                    pnsl = slice(pnch * 128, pnch * 128 + 128)
                    nc.tensor.matmul(pM[:], wvha_t[pnb][:, pnsl],
                                     arma_t[pnb][:], start=True, stop=False)
                    nc.tensor.matmul(pM[:], wvhb_t[pnb][:, pnsl],
                                     armb_t[pnb][:], start=False, stop=False)
                    nc.tensor.matmul(
                        pM[:], ident8dr[:],
                        ppm8[:, 0:2].rearrange("n j i t -> n j (i t)"),
                        start=False, stop=False,
                        perf_mode=mybir.MatmulPerfMode.DoubleRow,
                        skip_group_check=True,
                    )
                    nc.tensor.matmul(
                        pM[:], ident8[:],
                        ppm8[:, 2].rearrange("n i t -> n (i t)"),
                        start=False, stop=True, skip_group_check=True,
                    )
                    pend[-1] = [gi - 1, pM, pnb, pnch, 1]

                if pend and pend[0][4] == 1 and (gi - pend[0][0] >= 2
                                                 or gi >= NTOT):
                    # evac of chunk gi-2 (ACT: two behind, so the evac's
                    # reduce-wait never delays the next dvs in ACT's queue)
                    _, pM2, pnb2, pnch2, _st = pend.pop(0)
                    nc.scalar.copy(outacc_t[pnb2][:, pnch2, :], pM2[:])
                    ob = 2 if pnb2 == NB - 1 and pnch2 >= NCH - 4 else OUTB
                    if pnch2 % ob == ob - 1:
                        c0 = pnch2 - (ob - 1)
                        nc.sync.dma_start(
                            out_d[pnb2, c0:pnch2 + 1].rearrange(
                                "c p f -> p c f"),
                            outacc_t[pnb2][:, c0:pnch2 + 1],
                        )

                if gi < NTOT:
                    # pm8 [128, 3(j), 3(i), T] fp8 = ts8 * dvs (DVE); both
                    # inputs were produced a window ago -- never stalls.
                    pm8 = p_work.tile([128, 3, 3, T], FP8, tag="pm8")
                    nc.vector.tensor_tensor(
                        pm8[:],
                        cb_t[gi % 2][:, :, 0:384].rearrange(
                            "n j (i t) -> n j i t", i=3),
                        dvs_t[gi % 4][:].unsqueeze(2).broadcast_to(
                            (128, 3, 3, T)),
                        mybir.AluOpType.mult,
                    )
                    pend.append([gi, pm8, nb, nch, 0])
                    if gi + 2 < NTOT:
                        ts8_chunk(gi + 2)
                    if gi + 3 < NTOT:
                        dv_chunk(gi + 3)

    nc.compile()
    return nc


def _rodrigues_feat(pose):
    # pose: [NB, T, JB, 3] float32 -> (R - I) flattened [NB, T, PF]
    aa = pose.astype(np.float32)
    angle = np.sqrt((aa * aa).sum(-1, keepdims=True))            # [NB,T,JB,1]
    axis = aa / np.maximum(angle, 1e-8)
    x, y, z = axis[..., 0], axis[..., 1], axis[..., 2]
    s = np.sin(angle[..., 0])[..., None, None]
    c = np.cos(angle[..., 0])[..., None, None]
    zero = np.zeros_like(x)
    K = np.stack([
        np.stack([zero, -z, y], axis=-1),
        np.stack([z, zero, -x], axis=-1),
        np.stack([-y, x, zero], axis=-1),
    ], axis=-2)
    outer = axis[..., :, None] * axis[..., None, :]
    I = np.eye(3, dtype=np.float32)
    R = c * I + s * K + (1.0 - c) * outer
    return (R - I).reshape(aa.shape[0], aa.shape[1], PF)


def _prep_core(c, pose_body, trans, betas, A, v_template, shapedirs, posedirs,
               lbs_weights):
    bs = slice(NB * c, NB * (c + 1))

    # zc = [pose_feature | betas] per (sample, t), packed into fp8 dual-pairs
    # (z, z+103); the phantom row z=205 is the zero at (102, u=1).
    pf = _rodrigues_feat(pose_body[bs].reshape(NB, T, JB, 3))    # [NB,T,PF]
    zc = np.concatenate([pf, betas[bs]], axis=2)                 # [NB,T,Z]
    zcT = np.ascontiguousarray(zc.transpose(2, 0, 1))            # [Z,NB,T]
    zc8 = np.zeros((ZP, 2, NB, T), np.float32)
    zc8[:, 0] = zcT[0:ZP]
    zc8[0:Z - ZP, 1] = zcT[ZP:Z]
    zc8 = zc8.astype(NPF8)

    wt = np.concatenate(
        [lbs_weights[bs].transpose(0, 2, 1),
         np.ones((NB, 1, N), np.float32)], axis=1)             # [NB, 53, N]
    vth = np.concatenate(
        [v_template[bs], np.ones((NB, N, 1), np.float32)], axis=2)  # [NB,N,4]
    wvh = (vth.transpose(0, 2, 1)[:, :, None, :] * wt[:, None, :, :]
           ).reshape(NB, KA, N)                                # [NB,(j,k),N]
    wvha = np.ascontiguousarray(wvh[:, 0:128].transpose(1, 0, 2)).astype(NPBF16)
    wvhb = np.ascontiguousarray(wvh[:, 128:KA].transpose(1, 0, 2)).astype(NPBF16)

    arm = np.zeros((NB, 4, J + 1, 3, T), np.float32)
    arm[:, :, :J] = A[bs, :, :, 0:3, :].transpose(0, 4, 2, 3, 1)  # [nb,j,k,i,t]
    arm[:, 3, J] = trans[bs].transpose(0, 2, 1)                   # [nb,i,t]
    arm = arm.reshape(NB, KA, 3 * T)
    arma = np.ascontiguousarray(arm[:, 0:128].transpose(1, 0, 2)).astype(NPBF16)
    armb = np.ascontiguousarray(arm[:, 128:KA].transpose(1, 0, 2)).astype(NPBF16)

    wt8 = np.empty((26, 2, NB, N), np.float32)
    wt8[:, 0] = wt[:, 0:26].transpose(1, 0, 2)
    wt8[:, 1] = wt[:, 26:52].transpose(1, 0, 2)
    wt8 = wt8.astype(NPF8)

    ar8f = A[bs, :, :, 0:3, 0:3].transpose(0, 2, 4, 3, 1)      # [nb,k,j,i,t]
    ar8 = np.empty((26, 2, NB, 3, 3, T), np.float32)
    ar8[:, 0] = ar8f[:, 0:26].transpose(1, 0, 2, 3, 4)
    ar8[:, 1] = ar8f[:, 26:52].transpose(1, 0, 2, 3, 4)
    ar8 = ar8.astype(NPF8)

    D = np.concatenate([
        posedirs[bs].reshape(NB, PF, N, 3),
        shapedirs[bs].transpose(0, 3, 1, 2),                   # [NB, L, N, 3]
    ], axis=1)                                                 # [NB, Z, N, 3]
    Dt = D.transpose(0, 1, 3, 2) * DS                          # [NB, Z, 3, N]
    d8 = np.zeros((NB, ZP, 2, 3, N), np.float32)
    d8[:, :, 0] = Dt[:, 0:ZP]
    d8[:, 0:Z - ZP, 1] = Dt[:, ZP:Z]
    d8 = d8.astype(NPF8)

    return {
        "zc8": zc8, "wvha": wvha, "wvhb": wvhb,
        "arma": arma, "armb": armb, "wt8": wt8, "ar8": ar8, "d8": d8,
    }


def kernel(pose_body, trans, betas, A, v_template, shapedirs, posedirs,
           lbs_weights):
    if "nc" not in _CACHED:
        _CACHED["nc"] = _build_nc()
    nc = _CACHED["nc"]

    args = (pose_body, trans, betas, A, v_template, shapedirs, posedirs,
            lbs_weights)
    args = tuple(np.asarray(a, dtype=np.float32) for a in args)
    in_maps = [_prep_core(c, *args) for c in range(NCORES)]

    res = bass_utils.run_bass_kernel_spmd(nc, in_maps,
                                          core_ids=list(range(NCORES)))

    # out [NB, NCH, 128, 3*T] per core -> (B, T, N, 3)
    full = np.stack(
        [res.results[c]["out"].astype(np.float32) for c in range(NCORES)]
    )
    full = full.reshape(B, NCH, 128, 3, T).transpose(0, 4, 1, 2, 3)
    return np.ascontiguousarray(full.reshape(B, T, N, 3).astype(np.float32))
